# revision 31
# baseline (speedup 1.0000x reference)
"""Trainium2 Bass kernel for a binarized (1w1a) BasicBlock:

    out = relu(bn2(conv2(sign(pad(relu(bn1(conv1(sign(pad(x)), sign(w1)))))), sign(w2))) + x)

with 2x3 convs, C=256, B=64, H=W=32, pad = (W: 1 left/right, H: 1 bottom).

Strategy: data-parallel over batch across 8 NeuronCores (8 images/core).
Per core each conv is an implicit GEMM: input channels on the 128 SBUF
partitions, contraction over all 256 channels in one fp8e4 DoubleRow pass
(binarized +-1/0 exact in fp8; fp32 PSUM sums exact).

v2 layout: ALL 8 images of a core live in ONE contiguous "shared-pad" plane
per channel-tile: each padded row is 33 wide (32 data + 1 zero column that is
row h's right pad and row h+1's left pad), images separated by a 33-cell zero
row (img i's bottom pad), plus one global leading zero.  Every 2x3 tap is a
single offset into this stream, so a conv is 6 PSUM-accumulated matmuls per
512-column chunk (512 = one PSUM bank), 17 chunks x 2 output-channel tiles
per conv.  All epilogue ops are fully contiguous 512-wide:
  conv1: tensor_scalar (psum*inv1) is_gt (-bias1) -> {0,1} fp8 straight into
         conv2's input plane (pad cells re-zeroed by small gpsimd memsets);
  conv2: scalar_tensor_tensor (psum*inv2 + x_plane[bf16]) -> Relu+bias2
         activation (scalar engine) -> bf16 output plane.
The only 33->32 re-pitch happens in the output DMA (strided source).
Residual x and the output travel as bf16 (tolerance 2e-2 >> bf16's 0.4%).
"""

import numpy as np
import ml_dtypes

import concourse.mybir as mybir
import concourse.tile as tile
from concourse import bacc
from concourse.bass_utils import run_bass_kernel_spmd

N_CORES = 8
B, C, H, W = 64, 256, 32, 32
BL = B // N_CORES          # images per core
P = 128
KT = C // P                # channel tiles (contraction / output)
NPOS = 6                   # 2x3 kernel taps
EPS = 1e-5

PITCH = 33                 # padded row width (32 data + shared zero col)
IMGC = H * PITCH + PITCH   # cells per image incl bottom pad row = 1089
LEAD = 1                   # one global leading zero (left pad of img0 row0)
NCH = 17                   # 512-col chunks per kt-plane
CHW = 512                  # chunk width = one PSUM bank of f32
NSTREAM = 8752             # >= LEAD + BL*IMGC + max tap offset (35), %16 == 0
MAXOFF = PITCH + 2         # largest tap offset (kh=1, kw=2)
OSTREAM = 8720             # output plane: >= stream end + 1, %16 == 0
REG = 560                  # conv1 input region width: CHW + MAXOFF pad, %16 == 0
# chunk list: (start, width); last 512 split in two to shorten the tail
CHUNKS = [(i * CHW, CHW) for i in range(NCH - 1)] + [(8192, 256), (8448, 256)]
# output-plane DMA slices (start, end, last chunk index they depend on)
OSL = ((0, 1537, 2), (1537, 3073, 5), (3073, 4609, 8), (4609, 6145, 11),
       (6145, 7681, 14), (7681, 8193, 15), (8193, 8449, 16), (8449, 8720, 17))

F32 = mybir.dt.float32
BF16 = mybir.dt.bfloat16
FP8 = mybir.dt.float8e4
DR = mybir.MatmulPerfMode.DoubleRow

_CACHE = {}


def _img_base(b):
    return LEAD + b * IMGC


def _build():
    if "nc" in _CACHE:
        return _CACHE["nc"]

    nc = bacc.Bacc("TRN2", target_bir_lowering=False, debug=False)

    xq_d = nc.dram_tensor("xq1", [P, NCH, KT, REG], FP8, kind="ExternalInput")
    xp_d = nc.dram_tensor("xp", [P, KT, NSTREAM], BF16, kind="ExternalInput")
    w1_d = nc.dram_tensor("w1t", [KT, P, KT, NPOS, P], FP8, kind="ExternalInput")
    w2_d = nc.dram_tensor("w2t", [KT, P, KT, NPOS, P], FP8, kind="ExternalInput")
    bnv_d = nc.dram_tensor("bnv", [4, C], F32, kind="ExternalInput")
    out_d = nc.dram_tensor("out", [P, KT, OSTREAM], BF16, kind="ExternalOutput")

    # psum stream position q holds the conv value for plane cell q + 1 (the
    # global leading zero supplies the kw-1 left-pad shift), so every
    # output-side slice is the chunk range shifted by +1.  The output stays
    # in plane layout all the way to DRAM; the host strips the pad cells.

    with tile.TileContext(nc) as tc:
        with (
            tc.tile_pool(name="res", bufs=1) as res,
            tc.tile_pool(name="tmp", bufs=4) as tmp,
            tc.tile_pool(name="ps", bufs=6, space="PSUM") as ps,
        ):
            # PE warm-up while inputs land (HAM clock ramp)
            wu = res.tile([P, 512], FP8, tag="wu", name="wu")
            nc.vector.memset(wu[:], 0.0)
            wups = ps.tile([P, 512], F32, tag="wups", name="wups", bufs=1)
            for _ in range(6):
                nc.tensor.matmul(wups[:], wu[:, 0:P], wu[:], start=True, stop=True)

            # conv1 input: per-chunk regions so each chunk's matmul read-span
            # is exactly one region (precise deps); region groups stream in
            # consumption order, split between the two HWDGE queues
            xq1 = res.tile([P, NCH, KT, REG], FP8, tag="xq1", name="xq1")
            nc.sync.dma_start(xq1[:, 0:2], xq_d.ap()[:, 0:2])

            # weights on the scalar queue (w1 mt0 first -- gates conv1);
            # tiny BN vector rides the slow gpsimd queue
            w1sb, w2sb = [None, None], [None, None]
            for mt in range(KT):
                w1sb[mt] = res.tile([P, KT, NPOS, P], FP8, tag=f"w1q{mt}", name=f"w1q{mt}")
                nc.scalar.dma_start(w1sb[mt][:], w1_d.ap()[mt])
            bnsb = res.tile([P, 4 * KT], F32, tag="bnv", name="bnv")
            nc.gpsimd.dma_start(bnsb[:], bnv_d.ap().rearrange("v (t p) -> p (v t)", p=P))
            nc.scalar.dma_start(xq1[:, 2:6], xq_d.ap()[:, 2:6])
            nc.sync.dma_start(xq1[:, 6:12], xq_d.ap()[:, 6:12])
            nc.sync.dma_start(xq1[:, 12:NCH], xq_d.ap()[:, 12:NCH])

            inv1sb = bnsb[:, 0 * KT:1 * KT]
            nb1sb = bnsb[:, 1 * KT:2 * KT]
            inv2sb = bnsb[:, 2 * KT:3 * KT]
            b2sb = bnsb[:, 3 * KT:4 * KT]

            # conv2 input plane; tail cells never touched by epilogue chunks
            xq2 = res.tile([P, KT, NSTREAM], FP8, tag="xq2", name="xq2")
            nc.gpsimd.memset(xq2[:, :, NCH * CHW:NSTREAM], 0.0)

            for mt in range(KT):
                w2sb[mt] = res.tile([P, KT, NPOS, P], FP8, tag=f"w2q{mt}", name=f"w2q{mt}")
                nc.scalar.dma_start(w2sb[mt][:], w2_d.ap()[mt])

            # residual x (+ its plane holes = 0) as bf16, behind the rest
            xp = res.tile([P, KT, NSTREAM], BF16, tag="xp", name="xp")
            XPL = (0, 4384, NSTREAM)
            for i in range(2):
                (nc.sync if i % 2 else nc.scalar).dma_start(
                    xp[:, :, XPL[i]:XPL[i + 1]], xp_d.ap()[:, :, XPL[i]:XPL[i + 1]])

            # bf16 output plane, DMAed to DRAM in contiguous column slices
            ob = res.tile([P, KT, OSTREAM], BF16, tag="ob", name="ob")
            nc.gpsimd.memset(ob[:, :, 0:1], 0.0)                      # lead cell
            nc.gpsimd.memset(ob[:, :, NCH * CHW + 1:OSTREAM], 0.0)    # tail

            def conv_chunk(ci, mt, wsb, rhs_slicer):
                s, w = CHUNKS[ci]
                pt = ps.tile([P, CHW], F32, tag="ps", name=f"ps_{id(wsb)}_{ci}_{mt}")
                for pos in range(NPOS):
                    kh, kw = divmod(pos, 3)
                    off = kh * PITCH + kw
                    nc.tensor.matmul(
                        pt[:, 0:w],
                        wsb[mt][:, :, pos, :],
                        rhs_slicer(s, off, w),
                        start=(pos == 0),
                        stop=(pos == NPOS - 1),
                        perf_mode=DR,
                    )
                return pt

            def xq1_rhs(s, off, w):
                reg = min(s // CHW, NCH - 1)
                rel = s - reg * CHW
                return xq1[:, reg, :, rel + off: rel + off + w]

            def xq2_rhs(s, off, w):
                return xq2[:, :, s + off: s + off + w]

            def pad_fix(b):
                for kt in range(KT):
                    v = xq2[:, kt, _img_base(b):_img_base(b) + H * PITCH]
                    nc.gpsimd.memset(
                        v.rearrange("c (h w) -> c h w", w=PITCH)[:, :, W:PITCH], 0.0)
                    nc.gpsimd.memset(
                        xq2[:, kt, _img_base(b) + H * PITCH:_img_base(b + 1)], 0.0)

            # ---- conv1 + binarize epilogue (all contiguous) ----
            for ci, (s, w) in enumerate(CHUNKS):
                for mt in range(KT):
                    pt = conv_chunk(ci, mt, w1sb, xq1_rhs)
                    nc.vector.tensor_scalar(
                        xq2[:, mt, s + 1: s + w + 1],
                        pt[:, 0:w],
                        inv1sb[:, mt:mt + 1],
                        nb1sb[:, mt:mt + 1],
                        mybir.AluOpType.mult,
                        mybir.AluOpType.is_gt,
                    )
                # re-zero pad cells of any image fully covered by now
                done = (s + w + 1 - LEAD) // IMGC   # images fully written
                prev = (s + 1 - LEAD) // IMGC if ci else 0
                for b in range(prev, min(done, BL)):
                    pad_fix(b)
                if ci == 0:
                    nc.gpsimd.memset(xq2[:, :, 0:LEAD], 0.0)
            # images whose cells extend past the last chunk boundary
            se, swd = CHUNKS[-1]
            for b in range(max(0, (se + swd + 1 - LEAD) // IMGC), BL):
                pad_fix(b)

            # ---- conv2 + bn2 + residual + relu ----
            OUTQ = (nc.sync, nc.gpsimd)
            nslice = 0
            for ci, (s, w) in enumerate(CHUNKS):
                for mt in range(KT):
                    pt = conv_chunk(ci, mt, w2sb, xq2_rhs)
                    tt = tmp.tile([P, CHW], F32, tag="t2", name=f"t2_{ci}_{mt}")
                    nc.vector.scalar_tensor_tensor(
                        tt[:, 0:w],
                        pt[:, 0:w],
                        inv2sb[:, mt:mt + 1],
                        xp[:, mt, s + 1: s + w + 1],
                        mybir.AluOpType.mult,
                        mybir.AluOpType.add,
                    )
                    nc.scalar.activation(
                        ob[:, mt, s + 1: s + w + 1],
                        tt[:, 0:w],
                        mybir.ActivationFunctionType.Relu,
                        bias=b2sb[:, mt:mt + 1],
                        scale=1.0,
                    )
                # flush finished output-plane slices (contiguous, full-rate)
                while nslice < len(OSL) and OSL[nslice][2] == ci:
                    os_, oe, _ = OSL[nslice]
                    q = nc.sync if nslice >= len(OSL) - 2 else OUTQ[nslice % 2]
                    q.dma_start(out_d.ap()[:, :, os_:oe], ob[:, :, os_:oe])
                    nslice += 1

    nc.compile()
    _CACHE["nc"] = nc
    return nc


def _prep(w1, w2, gamma1, beta1, mean1, var1, gamma2, beta2, mean2, var2):
    """Host-side: fold BN, binarize + lay out weights as DoubleRow lhsT."""
    def fold(gamma, beta, mean, var):
        inv = (gamma.astype(np.float64) / np.sqrt(var.astype(np.float64) + EPS))
        inv = inv.astype(np.float32)
        bias = (beta.astype(np.float32) - mean.astype(np.float32) * inv)
        return inv, bias

    inv1, bias1 = fold(gamma1, beta1, mean1, var1)
    inv2, bias2 = fold(gamma2, beta2, mean2, var2)

    def wt(w):
        # [O, I, 2, 3] -> DoubleRow lhsT layout [mt, ci, ko, pos, co']
        s = np.sign(w).astype(np.float32)
        arr = s.transpose(1, 2, 3, 0).reshape(KT, P, NPOS, KT, P)  # [ko,ci,pos,mt,co']
        arr = arr.transpose(3, 1, 0, 2, 4)
        return np.ascontiguousarray(arr).astype(mybir.dt.np(FP8))

    bnv = np.ascontiguousarray(np.stack([inv1, -bias1, inv2, bias2]))
    return wt(w1), wt(w2), bnv


# global-plane columns of image interiors: cell(b, h, w) = LEAD + b*IMGC + h*PITCH + w
_INT_COLS = (
    LEAD
    + (np.arange(BL)[:, None, None] * IMGC)
    + (np.arange(H)[None, :, None] * PITCH)
    + np.arange(W)[None, None, :]
).ravel()


def _unpack_out(plane):
    """[P, KT, OSTREAM] bf16 output plane -> [BL, C, H, W] f32."""
    v = np.asarray(plane, dtype=np.float32)[:, :, _INT_COLS]      # [P, KT, BL*H*W]
    v = v.reshape(P, KT, BL, H * W).transpose(2, 1, 0, 3)         # [BL, KT, P, HW]
    return np.ascontiguousarray(v).reshape(BL, C, H, W)


def _in_maps(x, w1t, w2t, bnv):
    """Per-core inputs: xq1 = sign(x) in the fp8 global shared-pad plane
    [p, kt, NSTREAM]; xp = bf16 residual in the same plane (holes = 0)."""
    maps = []
    for cidx in range(N_CORES):
        xs = x[cidx * BL:(cidx + 1) * BL]                 # [BL, C, H, W]
        xh = np.ascontiguousarray(
            xs.reshape(BL, KT, P, H * W).transpose(2, 1, 0, 3))  # [P, KT, BL, HW]
        plane = np.zeros((P, KT, NSTREAM), np.float32)
        plane[:, :, _INT_COLS] = np.sign(xh).reshape(P, KT, BL * H * W)
        planeq = plane.astype(mybir.dt.np(FP8))
        xq = np.empty((P, NCH, KT, REG), mybir.dt.np(FP8))
        for c in range(NCH):
            xq[:, c] = planeq[:, :, c * CHW: c * CHW + REG]
        xplane = np.zeros((P, KT, NSTREAM), np.float32)
        xplane[:, :, _INT_COLS] = xh.reshape(P, KT, BL * H * W)
        xp = xplane.astype(ml_dtypes.bfloat16)
        maps.append({"xq1": xq, "xp": xp, "w1t": w1t, "w2t": w2t, "bnv": bnv})
    return maps


def kernel(x, w1, gamma1, beta1, mean1, var1,
           w2, gamma2, beta2, mean2, var2):
    x = np.asarray(x, dtype=np.float32)
    w1t, w2t, bnv = _prep(
        np.asarray(w1), np.asarray(w2),
        np.asarray(gamma1), np.asarray(beta1), np.asarray(mean1), np.asarray(var1),
        np.asarray(gamma2), np.asarray(beta2), np.asarray(mean2), np.asarray(var2),
    )

    nc = _build()
    in_maps = _in_maps(x, w1t, w2t, bnv)

    res = run_bass_kernel_spmd(nc, in_maps, core_ids=list(range(N_CORES)))
    out = np.concatenate([_unpack_out(r["out"]) for r in res.results], axis=0)
    return out


# revision 34
# speedup vs baseline: 1.0140x; 1.0140x over previous
"""Trainium2 Bass kernel for a binarized (1w1a) BasicBlock:

    out = relu(bn2(conv2(sign(pad(relu(bn1(conv1(sign(pad(x)), sign(w1)))))), sign(w2))) + x)

with 2x3 convs, C=256, B=64, H=W=32, pad = (W: 1 left/right, H: 1 bottom).

Strategy: data-parallel over batch across 8 NeuronCores (8 images/core).
Per core each conv is an implicit GEMM: input channels on the 128 SBUF
partitions, contraction over all 256 channels in one fp8e4 DoubleRow pass
(binarized +-1/0 exact in fp8; fp32 PSUM sums exact).

v2 layout: ALL 8 images of a core live in ONE contiguous "shared-pad" plane
per channel-tile: each padded row is 33 wide (32 data + 1 zero column that is
row h's right pad and row h+1's left pad), images separated by a 33-cell zero
row (img i's bottom pad), plus one global leading zero.  Every 2x3 tap is a
single offset into this stream, so a conv is 6 PSUM-accumulated matmuls per
512-column chunk (512 = one PSUM bank), 17 chunks x 2 output-channel tiles
per conv.  All epilogue ops are fully contiguous 512-wide:
  conv1: tensor_scalar (psum*inv1) is_gt (-bias1) -> {0,1} fp8 straight into
         conv2's input plane (pad cells re-zeroed by small gpsimd memsets);
  conv2: scalar_tensor_tensor (psum*inv2 + x_plane[bf16]) -> Relu+bias2
         activation (scalar engine) -> bf16 output plane.
The only 33->32 re-pitch happens in the output DMA (strided source).
Residual x and the output travel as bf16 (tolerance 2e-2 >> bf16's 0.4%).
"""

import numpy as np
import ml_dtypes

import concourse.mybir as mybir
import concourse.tile as tile
from concourse import bacc
from concourse.bass_utils import run_bass_kernel_spmd

N_CORES = 8
B, C, H, W = 64, 256, 32, 32
BL = B // N_CORES          # images per core
P = 128
KT = C // P                # channel tiles (contraction / output)
NPOS = 6                   # 2x3 kernel taps
EPS = 1e-5

PITCH = 33                 # padded row width (32 data + shared zero col)
IMGC = H * PITCH + PITCH   # cells per image incl bottom pad row = 1089
LEAD = 1                   # one global leading zero (left pad of img0 row0)
NCH = 17                   # 512-col chunks per kt-plane
CHW = 512                  # chunk width = one PSUM bank of f32
NSTREAM = 8752             # >= LEAD + BL*IMGC + max tap offset (35), %16 == 0
MAXOFF = PITCH + 2         # largest tap offset (kh=1, kw=2)
OSTREAM = 8720             # output plane: >= stream end + 1, %16 == 0
REG = 560                  # conv1 input region width: CHW + MAXOFF pad, %16 == 0
# chunk list: (start, width); last 512 split in two to shorten the tail
CHUNKS = [(i * CHW, CHW) for i in range(NCH - 1)] + [(8192, 256), (8448, 256)]
# output-plane DMA slices (start, end, last chunk index they depend on)
OSL = ((0, 1537, 2), (1537, 3073, 5), (3073, 4609, 8), (4609, 6145, 11),
       (6145, 7681, 14), (7681, 8193, 15), (8193, 8449, 16), (8449, 8720, 17))

F32 = mybir.dt.float32
BF16 = mybir.dt.bfloat16
FP8 = mybir.dt.float8e4
DR = mybir.MatmulPerfMode.DoubleRow

_CACHE = {}


def _img_base(b):
    return LEAD + b * IMGC


def _build():
    if "nc" in _CACHE:
        return _CACHE["nc"]

    nc = bacc.Bacc("TRN2", target_bir_lowering=False, debug=False)

    xq_d = nc.dram_tensor("xq1", [P, NCH, KT, REG], FP8, kind="ExternalInput")
    xp_d = nc.dram_tensor("xp", [P, KT, NSTREAM], BF16, kind="ExternalInput")
    w1_d = nc.dram_tensor("w1t", [KT, P, KT, NPOS, P], FP8, kind="ExternalInput")
    w2_d = nc.dram_tensor("w2t", [KT, P, KT, NPOS, P], FP8, kind="ExternalInput")
    bnv_d = nc.dram_tensor("bnv", [4, C], F32, kind="ExternalInput")
    out_d = nc.dram_tensor("out", [P, KT, OSTREAM], BF16, kind="ExternalOutput")

    # psum stream position q holds the conv value for plane cell q + 1 (the
    # global leading zero supplies the kw-1 left-pad shift), so every
    # output-side slice is the chunk range shifted by +1.  The output stays
    # in plane layout all the way to DRAM; the host strips the pad cells.

    with tile.TileContext(nc) as tc:
        with (
            tc.tile_pool(name="res", bufs=1) as res,
            tc.tile_pool(name="tmp", bufs=4) as tmp,
            tc.tile_pool(name="ps", bufs=7, space="PSUM") as ps,
        ):
            # PE warm-up while inputs land (HAM clock ramp)
            wu = res.tile([P, 512], FP8, tag="wu", name="wu")
            nc.vector.memset(wu[:], 0.0)
            wups = ps.tile([P, 512], F32, tag="wups", name="wups", bufs=1)
            for _ in range(6):
                nc.tensor.matmul(wups[:], wu[:, 0:P], wu[:], start=True, stop=True)

            # conv1 input: per-chunk regions so each chunk's matmul read-span
            # is exactly one region (precise deps); region groups stream on
            # the sync queue in consumption order
            xq1 = res.tile([P, NCH, KT, REG], FP8, tag="xq1", name="xq1")
            XGRP = ((0, 2), (2, 5), (5, 9), (9, 13), (13, NCH))
            nc.sync.dma_start(xq1[:, 0:2], xq_d.ap()[:, 0:2])

            # weights on the scalar queue, one DMA per conv (mt on the free
            # dim); tiny BN vector rides the slow gpsimd queue
            w1full = res.tile([P, KT, KT, NPOS, P], FP8, tag="w1q", name="w1q")
            nc.scalar.dma_start(
                w1full[:], w1_d.ap().rearrange("mt ci ko pos co -> ci mt ko pos co"))
            w1sb = [w1full[:, mt] for mt in range(KT)]
            bnsb = res.tile([P, 4 * KT], F32, tag="bnv", name="bnv")
            nc.gpsimd.dma_start(bnsb[:], bnv_d.ap().rearrange("v (t p) -> p (v t)", p=P))
            for lo, hi in XGRP[1:]:
                nc.sync.dma_start(xq1[:, lo:hi], xq_d.ap()[:, lo:hi])

            inv1sb = bnsb[:, 0 * KT:1 * KT]
            nb1sb = bnsb[:, 1 * KT:2 * KT]
            inv2sb = bnsb[:, 2 * KT:3 * KT]
            b2sb = bnsb[:, 3 * KT:4 * KT]

            # conv2 input plane; tail cells never touched by epilogue chunks
            xq2 = res.tile([P, KT, NSTREAM], FP8, tag="xq2", name="xq2")
            nc.gpsimd.memset(xq2[:, :, NCH * CHW:NSTREAM], 0.0)

            w2full = res.tile([P, KT, KT, NPOS, P], FP8, tag="w2q", name="w2q")
            nc.scalar.dma_start(
                w2full[:], w2_d.ap().rearrange("mt ci ko pos co -> ci mt ko pos co"))
            w2sb = [w2full[:, mt] for mt in range(KT)]

            # residual x (+ its plane holes = 0) as bf16, behind the rest
            xp = res.tile([P, KT, NSTREAM], BF16, tag="xp", name="xp")
            XPL = (0, 4384, NSTREAM)
            for i in range(2):
                (nc.sync if i % 2 else nc.scalar).dma_start(
                    xp[:, :, XPL[i]:XPL[i + 1]], xp_d.ap()[:, :, XPL[i]:XPL[i + 1]])

            # bf16 output plane, DMAed to DRAM in contiguous column slices
            ob = res.tile([P, KT, OSTREAM], BF16, tag="ob", name="ob")
            nc.gpsimd.memset(ob[:, :, 0:1], 0.0)                      # lead cell
            nc.gpsimd.memset(ob[:, :, NCH * CHW + 1:OSTREAM], 0.0)    # tail

            def conv_chunk(ci, mt, wsb, rhs_slicer):
                s, w = CHUNKS[ci]
                pt = ps.tile([P, CHW], F32, tag="ps", name=f"ps_{id(wsb)}_{ci}_{mt}")
                for pos in range(NPOS):
                    kh, kw = divmod(pos, 3)
                    off = kh * PITCH + kw
                    nc.tensor.matmul(
                        pt[:, 0:w],
                        wsb[mt][:, :, pos, :],
                        rhs_slicer(s, off, w),
                        start=(pos == 0),
                        stop=(pos == NPOS - 1),
                        perf_mode=DR,
                    )
                return pt

            def xq1_rhs(s, off, w):
                reg = min(s // CHW, NCH - 1)
                rel = s - reg * CHW
                return xq1[:, reg, :, rel + off: rel + off + w]

            def xq2_rhs(s, off, w):
                return xq2[:, :, s + off: s + off + w]

            def pad_fix(b):
                for kt in range(KT):
                    v = xq2[:, kt, _img_base(b):_img_base(b) + H * PITCH]
                    nc.gpsimd.memset(
                        v.rearrange("c (h w) -> c h w", w=PITCH)[:, :, W:PITCH], 0.0)
                    nc.gpsimd.memset(
                        xq2[:, kt, _img_base(b) + H * PITCH:_img_base(b + 1)], 0.0)

            # ---- conv1 + binarize epilogue (all contiguous) ----
            for ci, (s, w) in enumerate(CHUNKS):
                for mt in range(KT):
                    pt = conv_chunk(ci, mt, w1sb, xq1_rhs)
                    nc.vector.tensor_scalar(
                        xq2[:, mt, s + 1: s + w + 1],
                        pt[:, 0:w],
                        inv1sb[:, mt:mt + 1],
                        nb1sb[:, mt:mt + 1],
                        mybir.AluOpType.mult,
                        mybir.AluOpType.is_gt,
                    )
                # re-zero pad cells of any image fully covered by now
                done = (s + w + 1 - LEAD) // IMGC   # images fully written
                prev = (s + 1 - LEAD) // IMGC if ci else 0
                for b in range(prev, min(done, BL)):
                    pad_fix(b)
                if ci == 0:
                    nc.gpsimd.memset(xq2[:, :, 0:LEAD], 0.0)
            # images whose cells extend past the last chunk boundary
            se, swd = CHUNKS[-1]
            for b in range(max(0, (se + swd + 1 - LEAD) // IMGC), BL):
                pad_fix(b)

            # ---- conv2 + bn2 + residual + relu ----
            OUTQ = (nc.sync, nc.gpsimd)
            nslice = 0
            for ci, (s, w) in enumerate(CHUNKS):
                for mt in range(KT):
                    pt = conv_chunk(ci, mt, w2sb, xq2_rhs)
                    tt = tmp.tile([P, CHW], F32, tag="t2", name=f"t2_{ci}_{mt}")
                    nc.vector.scalar_tensor_tensor(
                        tt[:, 0:w],
                        pt[:, 0:w],
                        inv2sb[:, mt:mt + 1],
                        xp[:, mt, s + 1: s + w + 1],
                        mybir.AluOpType.mult,
                        mybir.AluOpType.add,
                    )
                    nc.scalar.activation(
                        ob[:, mt, s + 1: s + w + 1],
                        tt[:, 0:w],
                        mybir.ActivationFunctionType.Relu,
                        bias=b2sb[:, mt:mt + 1],
                        scale=1.0,
                    )
                # flush finished output-plane slices (contiguous, full-rate)
                while nslice < len(OSL) and OSL[nslice][2] == ci:
                    os_, oe, _ = OSL[nslice]
                    q = nc.sync if nslice >= len(OSL) - 2 else OUTQ[nslice % 2]
                    q.dma_start(out_d.ap()[:, :, os_:oe], ob[:, :, os_:oe])
                    nslice += 1

    nc.compile()
    _CACHE["nc"] = nc
    return nc


def _prep(w1, w2, gamma1, beta1, mean1, var1, gamma2, beta2, mean2, var2):
    """Host-side: fold BN, binarize + lay out weights as DoubleRow lhsT."""
    def fold(gamma, beta, mean, var):
        inv = (gamma.astype(np.float64) / np.sqrt(var.astype(np.float64) + EPS))
        inv = inv.astype(np.float32)
        bias = (beta.astype(np.float32) - mean.astype(np.float32) * inv)
        return inv, bias

    inv1, bias1 = fold(gamma1, beta1, mean1, var1)
    inv2, bias2 = fold(gamma2, beta2, mean2, var2)

    def wt(w):
        # [O, I, 2, 3] -> DoubleRow lhsT layout [mt, ci, ko, pos, co']
        s = np.sign(w).astype(np.float32)
        arr = s.transpose(1, 2, 3, 0).reshape(KT, P, NPOS, KT, P)  # [ko,ci,pos,mt,co']
        arr = arr.transpose(3, 1, 0, 2, 4)
        return np.ascontiguousarray(arr).astype(mybir.dt.np(FP8))

    bnv = np.ascontiguousarray(np.stack([inv1, -bias1, inv2, bias2]))
    return wt(w1), wt(w2), bnv


# global-plane columns of image interiors: cell(b, h, w) = LEAD + b*IMGC + h*PITCH + w
_INT_COLS = (
    LEAD
    + (np.arange(BL)[:, None, None] * IMGC)
    + (np.arange(H)[None, :, None] * PITCH)
    + np.arange(W)[None, None, :]
).ravel()


def _unpack_out(plane):
    """[P, KT, OSTREAM] bf16 output plane -> [BL, C, H, W] f32."""
    v = np.asarray(plane, dtype=np.float32)[:, :, _INT_COLS]      # [P, KT, BL*H*W]
    v = v.reshape(P, KT, BL, H * W).transpose(2, 1, 0, 3)         # [BL, KT, P, HW]
    return np.ascontiguousarray(v).reshape(BL, C, H, W)


def _in_maps(x, w1t, w2t, bnv):
    """Per-core inputs: xq1 = sign(x) in the fp8 global shared-pad plane
    [p, kt, NSTREAM]; xp = bf16 residual in the same plane (holes = 0)."""
    maps = []
    for cidx in range(N_CORES):
        xs = x[cidx * BL:(cidx + 1) * BL]                 # [BL, C, H, W]
        xh = np.ascontiguousarray(
            xs.reshape(BL, KT, P, H * W).transpose(2, 1, 0, 3))  # [P, KT, BL, HW]
        plane = np.zeros((P, KT, NSTREAM), np.float32)
        plane[:, :, _INT_COLS] = np.sign(xh).reshape(P, KT, BL * H * W)
        planeq = plane.astype(mybir.dt.np(FP8))
        xq = np.empty((P, NCH, KT, REG), mybir.dt.np(FP8))
        for c in range(NCH):
            xq[:, c] = planeq[:, :, c * CHW: c * CHW + REG]
        xplane = np.zeros((P, KT, NSTREAM), np.float32)
        xplane[:, :, _INT_COLS] = xh.reshape(P, KT, BL * H * W)
        xp = xplane.astype(ml_dtypes.bfloat16)
        maps.append({"xq1": xq, "xp": xp, "w1t": w1t, "w2t": w2t, "bnv": bnv})
    return maps


def kernel(x, w1, gamma1, beta1, mean1, var1,
           w2, gamma2, beta2, mean2, var2):
    x = np.asarray(x, dtype=np.float32)
    w1t, w2t, bnv = _prep(
        np.asarray(w1), np.asarray(w2),
        np.asarray(gamma1), np.asarray(beta1), np.asarray(mean1), np.asarray(var1),
        np.asarray(gamma2), np.asarray(beta2), np.asarray(mean2), np.asarray(var2),
    )

    nc = _build()
    in_maps = _in_maps(x, w1t, w2t, bnv)

    res = run_bass_kernel_spmd(nc, in_maps, core_ids=list(range(N_CORES)))
    out = np.concatenate([_unpack_out(r["out"]) for r in res.results], axis=0)
    return out


# revision 35
# speedup vs baseline: 1.0276x; 1.0135x over previous
"""Trainium2 Bass kernel for a binarized (1w1a) BasicBlock:

    out = relu(bn2(conv2(sign(pad(relu(bn1(conv1(sign(pad(x)), sign(w1)))))), sign(w2))) + x)

with 2x3 convs, C=256, B=64, H=W=32, pad = (W: 1 left/right, H: 1 bottom).

Strategy: data-parallel over batch across 8 NeuronCores (8 images/core).
Per core each conv is an implicit GEMM: input channels on the 128 SBUF
partitions, contraction over all 256 channels in one fp8e4 DoubleRow pass
(binarized +-1/0 exact in fp8; fp32 PSUM sums exact).

v2 layout: ALL 8 images of a core live in ONE contiguous "shared-pad" plane
per channel-tile: each padded row is 33 wide (32 data + 1 zero column that is
row h's right pad and row h+1's left pad), images separated by a 33-cell zero
row (img i's bottom pad), plus one global leading zero.  Every 2x3 tap is a
single offset into this stream, so a conv is 6 PSUM-accumulated matmuls per
512-column chunk (512 = one PSUM bank), 17 chunks x 2 output-channel tiles
per conv.  All epilogue ops are fully contiguous 512-wide:
  conv1: tensor_scalar (psum*inv1) is_gt (-bias1) -> {0,1} fp8 straight into
         conv2's input plane (pad cells re-zeroed by small gpsimd memsets);
  conv2: scalar_tensor_tensor (psum*inv2 + x_plane[bf16]) -> Relu+bias2
         activation (scalar engine) -> bf16 output plane.
The only 33->32 re-pitch happens in the output DMA (strided source).
Residual x and the output travel as bf16 (tolerance 2e-2 >> bf16's 0.4%).
"""

import numpy as np
import ml_dtypes

import concourse.mybir as mybir
import concourse.tile as tile
from concourse import bacc
from concourse.bass_utils import run_bass_kernel_spmd

N_CORES = 8
B, C, H, W = 64, 256, 32, 32
BL = B // N_CORES          # images per core
P = 128
KT = C // P                # channel tiles (contraction / output)
NPOS = 6                   # 2x3 kernel taps
EPS = 1e-5

PITCH = 33                 # padded row width (32 data + shared zero col)
IMGC = H * PITCH + PITCH   # cells per image incl bottom pad row = 1089
LEAD = 1                   # one global leading zero (left pad of img0 row0)
NCH = 17                   # 512-col chunks per kt-plane
CHW = 512                  # chunk width = one PSUM bank of f32
NSTREAM = 8752             # >= LEAD + BL*IMGC + max tap offset (35), %16 == 0
MAXOFF = PITCH + 2         # largest tap offset (kh=1, kw=2)
OSTREAM = 8720             # output plane: >= stream end + 1, %16 == 0
REG = 560                  # conv1 input region width: CHW + MAXOFF pad, %16 == 0
# chunk list: (start, width); last 512 split in two to shorten the tail
CHUNKS = [(i * CHW, CHW) for i in range(NCH - 1)] + [(8192, 256), (8448, 256)]
# output-plane DMA slices (start, end, last chunk index they depend on)
OSL = ((0, 1537, 2), (1537, 3073, 5), (3073, 4609, 8), (4609, 6145, 11),
       (6145, 7681, 14), (7681, 8193, 15), (8193, 8449, 16), (8449, 8720, 17))

F32 = mybir.dt.float32
BF16 = mybir.dt.bfloat16
FP8 = mybir.dt.float8e4
DR = mybir.MatmulPerfMode.DoubleRow

_CACHE = {}


def _img_base(b):
    return LEAD + b * IMGC


def _build():
    if "nc" in _CACHE:
        return _CACHE["nc"]

    nc = bacc.Bacc("TRN2", target_bir_lowering=False, debug=False)

    xq_d = nc.dram_tensor("xq1", [P, NCH, KT, REG], FP8, kind="ExternalInput")
    xp_d = nc.dram_tensor("xp", [P, KT, NSTREAM], BF16, kind="ExternalInput")
    w1_d = nc.dram_tensor("w1t", [KT, P, KT, NPOS, P], FP8, kind="ExternalInput")
    w2_d = nc.dram_tensor("w2t", [KT, P, KT, NPOS, P], FP8, kind="ExternalInput")
    bnv_d = nc.dram_tensor("bnv", [4, C], F32, kind="ExternalInput")
    out_d = nc.dram_tensor("out", [P, KT, OSTREAM], BF16, kind="ExternalOutput")

    # psum stream position q holds the conv value for plane cell q + 1 (the
    # global leading zero supplies the kw-1 left-pad shift), so every
    # output-side slice is the chunk range shifted by +1.  The output stays
    # in plane layout all the way to DRAM; the host strips the pad cells.

    with tile.TileContext(nc) as tc:
        with (
            tc.tile_pool(name="res", bufs=1) as res,
            tc.tile_pool(name="tmp", bufs=4) as tmp,
            tc.tile_pool(name="ps", bufs=7, space="PSUM") as ps,
        ):
            # PE warm-up while inputs land (HAM clock ramp)
            wu = res.tile([P, 512], FP8, tag="wu", name="wu")
            nc.vector.memset(wu[:], 0.0)
            wups = ps.tile([P, 512], F32, tag="wups", name="wups", bufs=1)
            for _ in range(7):
                nc.tensor.matmul(wups[:], wu[:, 0:P], wu[:], start=True, stop=True)

            # conv1 input: per-chunk regions so each chunk's matmul read-span
            # is exactly one region (precise deps); region groups stream on
            # the sync queue in consumption order
            xq1 = res.tile([P, NCH, KT, REG], FP8, tag="xq1", name="xq1")
            XGRP = ((0, 2), (2, 5), (5, 9), (9, 13), (13, NCH))
            nc.sync.dma_start(xq1[:, 0:2], xq_d.ap()[:, 0:2])

            # weights on the scalar queue, one DMA per conv (mt on the free
            # dim); tiny BN vector rides the slow gpsimd queue
            w1full = res.tile([P, KT, KT, NPOS, P], FP8, tag="w1q", name="w1q")
            nc.scalar.dma_start(
                w1full[:], w1_d.ap().rearrange("mt ci ko pos co -> ci mt ko pos co"))
            w1sb = [w1full[:, mt] for mt in range(KT)]
            bnsb = res.tile([P, 4 * KT], F32, tag="bnv", name="bnv")
            nc.gpsimd.dma_start(bnsb[:], bnv_d.ap().rearrange("v (t p) -> p (v t)", p=P))
            for lo, hi in XGRP[1:]:
                nc.sync.dma_start(xq1[:, lo:hi], xq_d.ap()[:, lo:hi])

            inv1sb = bnsb[:, 0 * KT:1 * KT]
            nb1sb = bnsb[:, 1 * KT:2 * KT]
            inv2sb = bnsb[:, 2 * KT:3 * KT]
            b2sb = bnsb[:, 3 * KT:4 * KT]

            # conv2 input plane; tail cells never touched by epilogue chunks
            xq2 = res.tile([P, KT, NSTREAM], FP8, tag="xq2", name="xq2")
            nc.gpsimd.memset(xq2[:, :, NCH * CHW:NSTREAM], 0.0)

            w2full = res.tile([P, KT, KT, NPOS, P], FP8, tag="w2q", name="w2q")
            nc.scalar.dma_start(
                w2full[:], w2_d.ap().rearrange("mt ci ko pos co -> ci mt ko pos co"))
            w2sb = [w2full[:, mt] for mt in range(KT)]

            # residual x (+ its plane holes = 0) as bf16, behind the rest
            xp = res.tile([P, KT, NSTREAM], BF16, tag="xp", name="xp")
            XPL = (0, 4384, NSTREAM)
            for i in range(2):
                (nc.sync if i % 2 else nc.scalar).dma_start(
                    xp[:, :, XPL[i]:XPL[i + 1]], xp_d.ap()[:, :, XPL[i]:XPL[i + 1]])

            # bf16 output plane, DMAed to DRAM in contiguous column slices
            ob = res.tile([P, KT, OSTREAM], BF16, tag="ob", name="ob")
            nc.gpsimd.memset(ob[:, :, 0:1], 0.0)                      # lead cell
            nc.gpsimd.memset(ob[:, :, NCH * CHW + 1:OSTREAM], 0.0)    # tail

            def conv_chunk(ci, mt, wsb, rhs_slicer):
                s, w = CHUNKS[ci]
                pt = ps.tile([P, CHW], F32, tag="ps", name=f"ps_{id(wsb)}_{ci}_{mt}")
                for pos in range(NPOS):
                    kh, kw = divmod(pos, 3)
                    off = kh * PITCH + kw
                    nc.tensor.matmul(
                        pt[:, 0:w],
                        wsb[mt][:, :, pos, :],
                        rhs_slicer(s, off, w),
                        start=(pos == 0),
                        stop=(pos == NPOS - 1),
                        perf_mode=DR,
                    )
                return pt

            def xq1_rhs(s, off, w):
                reg = min(s // CHW, NCH - 1)
                rel = s - reg * CHW
                return xq1[:, reg, :, rel + off: rel + off + w]

            def xq2_rhs(s, off, w):
                return xq2[:, :, s + off: s + off + w]

            def pad_fix(b):
                for kt in range(KT):
                    v = xq2[:, kt, _img_base(b):_img_base(b) + H * PITCH]
                    nc.gpsimd.memset(
                        v.rearrange("c (h w) -> c h w", w=PITCH)[:, :, W:PITCH], 0.0)
                    nc.gpsimd.memset(
                        xq2[:, kt, _img_base(b) + H * PITCH:_img_base(b + 1)], 0.0)

            # ---- conv1 + binarize epilogue (all contiguous) ----
            for ci, (s, w) in enumerate(CHUNKS):
                for mt in range(KT):
                    pt = conv_chunk(ci, mt, w1sb, xq1_rhs)
                    nc.vector.tensor_scalar(
                        xq2[:, mt, s + 1: s + w + 1],
                        pt[:, 0:w],
                        inv1sb[:, mt:mt + 1],
                        nb1sb[:, mt:mt + 1],
                        mybir.AluOpType.mult,
                        mybir.AluOpType.is_gt,
                    )
                # re-zero pad cells of any image fully covered by now
                done = (s + w + 1 - LEAD) // IMGC   # images fully written
                prev = (s + 1 - LEAD) // IMGC if ci else 0
                for b in range(prev, min(done, BL)):
                    pad_fix(b)
                if ci == 0:
                    nc.gpsimd.memset(xq2[:, :, 0:LEAD], 0.0)
            # images whose cells extend past the last chunk boundary
            se, swd = CHUNKS[-1]
            for b in range(max(0, (se + swd + 1 - LEAD) // IMGC), BL):
                pad_fix(b)

            # ---- conv2 + bn2 + residual + relu ----
            OUTQ = (nc.sync, nc.gpsimd)
            nslice = 0
            for ci, (s, w) in enumerate(CHUNKS):
                for mt in range(KT):
                    pt = conv_chunk(ci, mt, w2sb, xq2_rhs)
                    tt = tmp.tile([P, CHW], F32, tag="t2", name=f"t2_{ci}_{mt}")
                    nc.vector.scalar_tensor_tensor(
                        tt[:, 0:w],
                        pt[:, 0:w],
                        inv2sb[:, mt:mt + 1],
                        xp[:, mt, s + 1: s + w + 1],
                        mybir.AluOpType.mult,
                        mybir.AluOpType.add,
                    )
                    nc.scalar.activation(
                        ob[:, mt, s + 1: s + w + 1],
                        tt[:, 0:w],
                        mybir.ActivationFunctionType.Relu,
                        bias=b2sb[:, mt:mt + 1],
                        scale=1.0,
                    )
                # flush finished output-plane slices (contiguous, full-rate)
                while nslice < len(OSL) and OSL[nslice][2] == ci:
                    os_, oe, _ = OSL[nslice]
                    q = nc.sync if nslice >= len(OSL) - 2 else OUTQ[nslice % 2]
                    q.dma_start(out_d.ap()[:, :, os_:oe], ob[:, :, os_:oe])
                    nslice += 1

    nc.compile()
    _CACHE["nc"] = nc
    return nc


def _prep(w1, w2, gamma1, beta1, mean1, var1, gamma2, beta2, mean2, var2):
    """Host-side: fold BN, binarize + lay out weights as DoubleRow lhsT."""
    def fold(gamma, beta, mean, var):
        inv = (gamma.astype(np.float64) / np.sqrt(var.astype(np.float64) + EPS))
        inv = inv.astype(np.float32)
        bias = (beta.astype(np.float32) - mean.astype(np.float32) * inv)
        return inv, bias

    inv1, bias1 = fold(gamma1, beta1, mean1, var1)
    inv2, bias2 = fold(gamma2, beta2, mean2, var2)

    def wt(w):
        # [O, I, 2, 3] -> DoubleRow lhsT layout [mt, ci, ko, pos, co']
        s = np.sign(w).astype(np.float32)
        arr = s.transpose(1, 2, 3, 0).reshape(KT, P, NPOS, KT, P)  # [ko,ci,pos,mt,co']
        arr = arr.transpose(3, 1, 0, 2, 4)
        return np.ascontiguousarray(arr).astype(mybir.dt.np(FP8))

    bnv = np.ascontiguousarray(np.stack([inv1, -bias1, inv2, bias2]))
    return wt(w1), wt(w2), bnv


# global-plane columns of image interiors: cell(b, h, w) = LEAD + b*IMGC + h*PITCH + w
_INT_COLS = (
    LEAD
    + (np.arange(BL)[:, None, None] * IMGC)
    + (np.arange(H)[None, :, None] * PITCH)
    + np.arange(W)[None, None, :]
).ravel()


def _unpack_out(plane):
    """[P, KT, OSTREAM] bf16 output plane -> [BL, C, H, W] f32."""
    v = np.asarray(plane, dtype=np.float32)[:, :, _INT_COLS]      # [P, KT, BL*H*W]
    v = v.reshape(P, KT, BL, H * W).transpose(2, 1, 0, 3)         # [BL, KT, P, HW]
    return np.ascontiguousarray(v).reshape(BL, C, H, W)


def _in_maps(x, w1t, w2t, bnv):
    """Per-core inputs: xq1 = sign(x) in the fp8 global shared-pad plane
    [p, kt, NSTREAM]; xp = bf16 residual in the same plane (holes = 0)."""
    maps = []
    for cidx in range(N_CORES):
        xs = x[cidx * BL:(cidx + 1) * BL]                 # [BL, C, H, W]
        xh = np.ascontiguousarray(
            xs.reshape(BL, KT, P, H * W).transpose(2, 1, 0, 3))  # [P, KT, BL, HW]
        plane = np.zeros((P, KT, NSTREAM), np.float32)
        plane[:, :, _INT_COLS] = np.sign(xh).reshape(P, KT, BL * H * W)
        planeq = plane.astype(mybir.dt.np(FP8))
        xq = np.empty((P, NCH, KT, REG), mybir.dt.np(FP8))
        for c in range(NCH):
            xq[:, c] = planeq[:, :, c * CHW: c * CHW + REG]
        xplane = np.zeros((P, KT, NSTREAM), np.float32)
        xplane[:, :, _INT_COLS] = xh.reshape(P, KT, BL * H * W)
        xp = xplane.astype(ml_dtypes.bfloat16)
        maps.append({"xq1": xq, "xp": xp, "w1t": w1t, "w2t": w2t, "bnv": bnv})
    return maps


def kernel(x, w1, gamma1, beta1, mean1, var1,
           w2, gamma2, beta2, mean2, var2):
    x = np.asarray(x, dtype=np.float32)
    w1t, w2t, bnv = _prep(
        np.asarray(w1), np.asarray(w2),
        np.asarray(gamma1), np.asarray(beta1), np.asarray(mean1), np.asarray(var1),
        np.asarray(gamma2), np.asarray(beta2), np.asarray(mean2), np.asarray(var2),
    )

    nc = _build()
    in_maps = _in_maps(x, w1t, w2t, bnv)

    res = run_bass_kernel_spmd(nc, in_maps, core_ids=list(range(N_CORES)))
    out = np.concatenate([_unpack_out(r["out"]) for r in res.results], axis=0)
    return out
